# revision 8
# baseline (speedup 1.0000x reference)
"""Trainium2 Bass kernel: 128-group Walsh-Hadamard transform.

Full input x: (4, 4096, 4096) fp32. Viewed as (524288, 128): each row is one
128-element group; output row = row @ (H_128 * 1/sqrt(128)), H_128 the
Sylvester-ordered Hadamard matrix (symmetric, entries +-1).

Sharding: pure data-parallel over 8 cores; each core handles 65536 rows
(32 MiB in / 32 MiB out).

Per-core pipeline (v3, fp16 PE path):
  SWDGE DMA in with fp32->fp16 cast (2 MiB chunks) -> PE transpose in fp16
  (group dim -> partitions), 4 sub-tiles batched per PSUM bank -> one DVE
  copy PSUM->SBUF per batch -> 4x PE matmul lhsT=Xt(f16), rhs=H(+-1 f16),
  fp32 accumulate into one PSUM bank -> one copy+scale (x 1/sqrt(128))
  PSUM->SBUF per batch, alternating DVE/ACT -> HWDGE DMA out (2 MiB chunks).

fp16 through the PE keeps weight loads on the fast-weight-load path and
matmuls at 1 cycle/row (fp32 would be 175 ns LDWEIGHTS + 2 half-rate
matmul passes per tile, which makes TensorE the bottleneck at ~220 us).
Input quantization to fp16 bounds rel err at ~5e-4.
"""

import numpy as np

import concourse.mybir as mybir
import concourse.bacc as bacc
from concourse.bass import Bass
from concourse.tile import TileContext
from concourse.bass_utils import run_bass_kernel_spmd

GROUP = 128
LOG2_N = 7
SCALE = 1.0 / np.sqrt(GROUP)
N_CORES = 8
FULL_SHAPE = (4, 4096, 4096)
R_TOTAL = 4 * 4096 * 4096 // GROUP  # 524288
R_CORE = R_TOTAL // N_CORES  # 65536

CH_ROWS = 2048  # rows per DMA chunk (1 MiB fp32)
RL = CH_ROWS // 128  # 128x128 sub-tiles per chunk (32)
NG = RL // 4  # groups of 4 sub-tiles (8)
NCH = R_CORE // CH_ROWS  # chunks per core (16)

F32 = mybir.dt.float32
F16 = mybir.dt.float16


def _hadamard128() -> np.ndarray:
    h = np.array([[1.0]], dtype=np.float32)
    for _ in range(LOG2_N):
        h = np.block([[h, h], [h, -h]]).astype(np.float32)
    return h


def _build_nc() -> Bass:
    nc = bacc.Bacc(None, target_bir_lowering=False)
    x_in = nc.declare_dram_parameter("x", [R_CORE, GROUP], F32, isOutput=False)
    h_in = nc.declare_dram_parameter("hmat", [GROUP, GROUP], F16, isOutput=False)
    i_in = nc.declare_dram_parameter("ident", [GROUP, GROUP], F16, isOutput=False)
    y_out = nc.declare_dram_parameter("out", [R_CORE, GROUP], F32, isOutput=True)

    # chunk view: row = c*CH_ROWS + p*RL + r,  partition dim = p
    xv = x_in.rearrange("(c p r) e -> c p (r e)", p=128, r=RL)
    yv = y_out.rearrange("(c p r) e -> c p (r e)", p=128, r=RL)

    with TileContext(nc) as tc:
        with (
            tc.tile_pool(name="const", bufs=1) as cpool,
            tc.tile_pool(name="xin", bufs=3) as xpool,
            tc.tile_pool(name="yout", bufs=3) as ypool,
            tc.tile_pool(name="xtsb", bufs=4) as xtpool,
            tc.tile_pool(name="pst", bufs=3, space="PSUM") as pst,
            tc.tile_pool(name="psy", bufs=3, space="PSUM") as psy,
        ):
            h_sb = cpool.tile([GROUP, GROUP], F16, tag="hmat")
            nc.sync.dma_start(out=h_sb, in_=h_in.ap())
            i_sb = cpool.tile([GROUP, GROUP], F16, tag="ident")
            nc.sync.dma_start(out=i_sb, in_=i_in.ap())

            for c in range(NCH):
                x_tile = xpool.tile([128, CH_ROWS], F16)
                nc.gpsimd.dma_start(out=x_tile, in_=xv[c])  # fp32 -> fp16 cast
                y_tile = ypool.tile([128, CH_ROWS], F32)
                for g in range(NG):
                    xt_ps = pst.tile([128, 512], F16)
                    for k in range(4):
                        rl = g * 4 + k
                        nc.tensor.transpose(
                            out=xt_ps[:, k * 128 : (k + 1) * 128],
                            in_=x_tile[:, rl * 128 : (rl + 1) * 128],
                            identity=i_sb,
                        )
                    xt_sb = xtpool.tile([128, 512], F16)
                    nc.vector.tensor_copy(out=xt_sb, in_=xt_ps)
                    y_ps = psy.tile([128, 512], F32)
                    for k in range(4):
                        nc.tensor.matmul(
                            out=y_ps[:, k * 128 : (k + 1) * 128],
                            lhsT=xt_sb[:, k * 128 : (k + 1) * 128],
                            rhs=h_sb,
                        )
                    ys = y_tile[:, g * 512 : (g + 1) * 512]
                    if g % 2 == 0:
                        nc.scalar.mul(ys, y_ps, float(SCALE))
                    else:
                        nc.vector.tensor_scalar_mul(ys, y_ps, float(SCALE))
                out_eng = nc.sync if c % 2 == 0 else nc.scalar
                out_eng.dma_start(out=yv[c], in_=y_tile)
    nc.compile()
    return nc


_CACHE: dict = {}


def _get_nc() -> Bass:
    if "nc" not in _CACHE:
        _CACHE["nc"] = _build_nc()
    return _CACHE["nc"]


def _run(x: np.ndarray, trace: bool = False):
    x = np.ascontiguousarray(x, dtype=np.float32).reshape(R_TOTAL, GROUP)
    hmat = _hadamard128().astype(np.float16)
    ident = np.eye(GROUP, dtype=np.float16)
    in_maps = [
        {
            "x": np.ascontiguousarray(x[i * R_CORE : (i + 1) * R_CORE]),
            "hmat": hmat,
            "ident": ident,
        }
        for i in range(N_CORES)
    ]
    nc = _get_nc()
    res = run_bass_kernel_spmd(nc, in_maps, list(range(N_CORES)), trace=trace)
    out = np.concatenate([r["out"] for r in res.results], axis=0)
    return out.reshape(FULL_SHAPE), res


def kernel(x: np.ndarray) -> np.ndarray:
    out, _ = _run(x, trace=False)
    return out


# revision 9
# speedup vs baseline: 1.1605x; 1.1605x over previous
"""Trainium2 Bass kernel: 128-group Walsh-Hadamard transform.

Full input x: (4, 4096, 4096) fp32. Viewed as (524288, 128): each row is one
128-element group; output row = row @ (H_128 * 1/sqrt(128)), H_128 the
Sylvester-ordered Hadamard matrix (symmetric, entries +-1).

Sharding: pure data-parallel over 8 cores; each core handles 65536 rows
(32 MiB in / 32 MiB out).

Per-core pipeline (v3, fp16 PE path):
  SWDGE DMA in with fp32->fp16 cast (2 MiB chunks) -> PE transpose in fp16
  (group dim -> partitions), 4 sub-tiles batched per PSUM bank -> one DVE
  copy PSUM->SBUF per batch -> 4x PE matmul lhsT=Xt(f16), rhs=H(+-1 f16),
  fp32 accumulate into one PSUM bank -> one copy+scale (x 1/sqrt(128))
  PSUM->SBUF per batch, alternating DVE/ACT -> HWDGE DMA out (2 MiB chunks).

fp16 through the PE keeps weight loads on the fast-weight-load path and
matmuls at 1 cycle/row (fp32 would be 175 ns LDWEIGHTS + 2 half-rate
matmul passes per tile, which makes TensorE the bottleneck at ~220 us).
Input quantization to fp16 bounds rel err at ~5e-4.
"""

import numpy as np

import concourse.mybir as mybir
import concourse.bacc as bacc
from concourse.bass import Bass
from concourse.tile import TileContext
from concourse.bass_utils import run_bass_kernel_spmd

GROUP = 128
LOG2_N = 7
SCALE = 1.0 / np.sqrt(GROUP)
N_CORES = 8
FULL_SHAPE = (4, 4096, 4096)
R_TOTAL = 4 * 4096 * 4096 // GROUP  # 524288
R_CORE = R_TOTAL // N_CORES  # 65536

CH_ROWS = 2048  # rows per DMA chunk (1 MiB fp32)
RL = CH_ROWS // 128  # 128x128 sub-tiles per chunk (32)
NG = RL // 4  # groups of 4 sub-tiles (8)
NCH = R_CORE // CH_ROWS  # chunks per core (16)

F32 = mybir.dt.float32
F16 = mybir.dt.float16


def _hadamard128() -> np.ndarray:
    h = np.array([[1.0]], dtype=np.float32)
    for _ in range(LOG2_N):
        h = np.block([[h, h], [h, -h]]).astype(np.float32)
    return h


def _build_nc() -> Bass:
    nc = bacc.Bacc(None, target_bir_lowering=False)
    x_in = nc.declare_dram_parameter("x", [R_CORE, GROUP], F32, isOutput=False)
    h_in = nc.declare_dram_parameter("hmat", [GROUP, GROUP], F16, isOutput=False)
    i_in = nc.declare_dram_parameter("ident", [GROUP, GROUP], F16, isOutput=False)
    y_out = nc.declare_dram_parameter("out", [R_CORE, GROUP], F32, isOutput=True)

    # chunk view: row = c*CH_ROWS + p*RL + r,  partition dim = p
    xv = x_in.rearrange("(c p r) e -> c p (r e)", p=128, r=RL)
    yv = y_out.rearrange("(c p r) e -> c p (r e)", p=128, r=RL)

    with TileContext(nc) as tc:
        with (
            tc.tile_pool(name="const", bufs=1) as cpool,
            tc.tile_pool(name="xin", bufs=3) as xpool,
            tc.tile_pool(name="yout", bufs=3) as ypool,
            tc.tile_pool(name="xtsb", bufs=4) as xtpool,
            tc.tile_pool(name="pst", bufs=3, space="PSUM") as pst,
            tc.tile_pool(name="psy", bufs=3, space="PSUM") as psy,
        ):
            h_sb = cpool.tile([GROUP, GROUP], F16, tag="hmat")
            nc.sync.dma_start(out=h_sb, in_=h_in.ap())
            i_sb = cpool.tile([GROUP, GROUP], F16, tag="ident")
            nc.sync.dma_start(out=i_sb, in_=i_in.ap())

            for c in range(NCH):
                x_tile = xpool.tile([128, CH_ROWS], F16)
                nc.gpsimd.dma_start(out=x_tile, in_=xv[c])  # fp32 -> fp16 cast
                y_tile = ypool.tile([128, CH_ROWS], F32)
                for g in range(NG):
                    xt_ps = pst.tile([128, 512], F16)
                    for k in range(4):
                        rl = g * 4 + k
                        nc.tensor.transpose(
                            out=xt_ps[:, k * 128 : (k + 1) * 128],
                            in_=x_tile[:, rl * 128 : (rl + 1) * 128],
                            identity=i_sb,
                        )
                    xt_sb = xtpool.tile([128, 512], F16)
                    nc.vector.tensor_copy(out=xt_sb, in_=xt_ps)
                    y_ps = psy.tile([128, 512], F32)
                    for k in range(4):
                        nc.tensor.matmul(
                            out=y_ps[:, k * 128 : (k + 1) * 128],
                            lhsT=xt_sb[:, k * 128 : (k + 1) * 128],
                            rhs=h_sb,
                        )
                    ys = y_tile[:, g * 512 : (g + 1) * 512]
                    if g % 2 == 0:
                        nc.scalar.mul(ys, y_ps, float(SCALE))
                    else:
                        nc.vector.tensor_scalar_mul(ys, y_ps, float(SCALE))
                nc.sync.dma_start(out=yv[c], in_=y_tile)
    nc.compile()
    return nc


_CACHE: dict = {}


def _get_nc() -> Bass:
    if "nc" not in _CACHE:
        _CACHE["nc"] = _build_nc()
    return _CACHE["nc"]


def _run(x: np.ndarray, trace: bool = False):
    x = np.ascontiguousarray(x, dtype=np.float32).reshape(R_TOTAL, GROUP)
    hmat = _hadamard128().astype(np.float16)
    ident = np.eye(GROUP, dtype=np.float16)
    in_maps = [
        {
            "x": np.ascontiguousarray(x[i * R_CORE : (i + 1) * R_CORE]),
            "hmat": hmat,
            "ident": ident,
        }
        for i in range(N_CORES)
    ]
    nc = _get_nc()
    res = run_bass_kernel_spmd(nc, in_maps, list(range(N_CORES)), trace=trace)
    out = np.concatenate([r["out"] for r in res.results], axis=0)
    return out.reshape(FULL_SHAPE), res


def kernel(x: np.ndarray) -> np.ndarray:
    out, _ = _run(x, trace=False)
    return out
